# revision 37
# baseline (speedup 1.0000x reference)
"""MLA (multi-head latent attention) Trainium2 kernel, 8-core SPMD.

Sharding: 2 head-groups x 4 query-row-groups grid over 8 NeuronCores.
  core c: gi = c % 2  -> heads [gi*16, gi*16+16)  (of H=32)
          ri = c // 2 -> query rows [ri*512, ri*512+512)  (of S=2048)
Each core computes a partial output  pout = O(its heads, its rows) @ w_out[rows of its heads]
in fp16; the host sums the two head-group partials per row block and adds b_out.

Precision: matmuls on error-attenuated paths (C_Q, Q/Qr projections, the
K-side latent, and the Q.K^T scores) run in fp8e4m3 with DoubleRow perf
mode (2 contraction tiles per PE pass); the V path, att.V, and the output
projection stay bf16 with fp32 PSUM so fp8 noise never reaches the output
unattenuated (measured end-to-end L2 rel err ~6e-3 vs the fp32 reference).
The softmax denominator is computed as a per-partition DVE tensor_reduce
over the exp tiles plus one 128-wide ones-matmul per head.  Softmax skips
max-subtraction: logits are bounded for this problem's scale.

NOTE: SWDGE (Pool-queue) DMA traffic concurrent with DoubleRow matmuls
crashes the PE exec unit; all streams ride the SP/ACT hardware DGE queues
and the Pool queue carries only the AllGather.

Self-contained: shapes/layouts hardcoded; host does layout/cast/shard,
device kernel does all matmul/softmax work, host sums 2 partials per row block.
"""

import numpy as np
import ml_dtypes

import jax
from jax.sharding import Mesh, PartitionSpec, NamedSharding
try:
    from jax.experimental.shard_map import shard_map
except ImportError:  # newer jax
    from jax import shard_map

import concourse.tile as tile
from concourse import bacc, mybir
from concourse import bass2jax

BF16 = mybir.dt.bfloat16
F32 = mybir.dt.float32
F16 = mybir.dt.float16
F8 = mybir.dt.float8e4
DR_MODE = mybir.MatmulPerfMode.DoubleRow
AFT = mybir.ActivationFunctionType
ALU = mybir.AluOpType

# problem dims
S, DE, DC1, DC, DR, H, DH, DM = 2048, 4096, 1536, 512, 64, 32, 128, 4096
NG, NR = 2, 4           # head groups x row groups = 8 cores
GH = H // NG            # 16 heads per core
QB = S // NR            # 512 query rows per core
SCALER = float(1.0 / np.sqrt(np.float32(DH + DR)))
P = 128


def _emit_body(nc, tc, t):
    """Emit one full iteration of the per-core computation.

    Phase order (chosen so the AllGather hides under independent PE work):
      B-shard: C_KVT/KrT for this core's 256-key slice  -> AllGather kickoff
      A:       C_QT (full, this core's 512 query rows)
      QT-all:  Q/Qr projections for all 16 heads (needs only C_QT)
      C:       per head: KT, scores^T, exp, AV, denominators
      D:       partial out-projection
    """
    from contextlib import ExitStack
    from concourse.tile_rust import add_dep_helper

    with ExitStack() as ctx:
        # PSUM pools: 3+2+1+2 = 8 banks exactly
        psg = ctx.enter_context(tc.tile_pool(name="psg", bufs=2, space="PSUM"))
        pss = ctx.enter_context(tc.tile_pool(name="pss", bufs=3, space="PSUM"))
        psd = ctx.enter_context(tc.tile_pool(name="psd", bufs=1, space="PSUM"))
        pso = ctx.enter_context(tc.tile_pool(name="pso", bufs=2, space="PSUM"))

        cpool = ctx.enter_context(tc.tile_pool(name="persist", bufs=1))
        pcw = ctx.enter_context(tc.tile_pool(name="pcw", bufs=2))
        qtp = ctx.enter_context(tc.tile_pool(name="qtp", bufs=1))
        cqt_cm = tc.tile_pool(name="cqt", bufs=1)
        cqtp = cqt_cm.__enter__()
        iop_cm = tc.tile_pool(name="iop", bufs=1)
        iop = iop_cm.__enter__()
        pa_cm = tc.tile_pool(name="ph_a", bufs=1)
        pa = pa_cm.__enter__()

        # ---------- DMA ordering helpers ----------
        gin_dma = [None]
        crit_dmas = []

        def after_crit(bass_inst, n=None):
            for d in (crit_dmas if n is None else crit_dmas[:n]):
                add_dep_helper(bass_inst.ins, d,
                               reason="defer until B-critical DMAs issued")
            return bass_inst

        def after_gin(bass_inst):
            if gin_dma[0] is not None:
                add_dep_helper(bass_inst.ins, gin_dma[0],
                               reason="defer until collective input sent")
            return bass_inst

        # ---------- B-critical loads ----------
        wdkv_chunks = []
        for ch in range(4):
            wch = iop.tile([P, 8, DC], BF16, tag="wdkv", bufs=2, name=f"wdkv{ch}")
            ins = nc.scalar.dma_start(wch[:], t["wdkv"][:, ch * 8:(ch + 1) * 8, :])
            crit_dmas.append(ins.ins)
            wdkv_chunks.append(wch)
        wrk = iop.tile([P, 32, DR], BF16, tag="wrk", name="wrk")
        crit_dmas.append(nc.scalar.dma_start(wrk[:], t["wrk"][:]).ins)
        # seqkb chunk 0 first, then wq0 (phase A's first weight) slotted in
        # before the rest so phase A can start the moment phase B's PE work
        # drains; seqmy chunks stream behind and are consumed in ko order
        seqkb = iop.tile([P, 32, 256], BF16, tag="seqkb", name="seqkb")
        wq0 = pa.tile([P, 32, 128], F8, tag="wdqq", bufs=2, name="wdqq0")
        seqmy = pa.tile([P, 32, QB], F8, tag="seqmy", name="seqmy")
        ins = nc.sync.dma_start(seqkb[:, 0:8, :], t["seqT_mykb"][:, 0:8, :])
        crit_dmas.append(ins.ins)
        nc.sync.dma_start(wq0[:], t["wdq"][0])
        for ch in range(1, 4):
            ins = nc.sync.dma_start(seqkb[:, ch * 8:(ch + 1) * 8, :],
                                    t["seqT_mykb"][:, ch * 8:(ch + 1) * 8, :])
            crit_dmas.append(ins.ins)
        for ch in range(8):
            nc.sync.dma_start(seqmy[:, ch * 4:(ch + 1) * 4, :],
                              t["seqT_my"][:, ch * 4:(ch + 1) * 4, :])
        last_wdqq = [None]

        # ---------- persistent tiles ----------
        C_KVT = cpool.tile([P, 4, S], BF16, tag="C_KVT", name="C_KVT")
        C_KVT8 = cpool.tile([P, 4, S], F8, tag="C_KVT8", name="C_KVT8")
        KrT = cpool.tile([P, S], BF16, tag="KrT", name="KrT")
        OT = cpool.tile([P, GH, QB], BF16, tag="OT", name="OT")
        ones128 = cpool.tile([P, P], BF16, tag="ones128", name="ones128")
        nc.any.memset(ones128[:], 1.0)
        bdq = cpool.tile([P, 12], F32, tag="bdq", name="bdq")
        bdkv = cpool.tile([P, 4], F32, tag="bdkv", name="bdkv")
        brk = cpool.tile([DR, 1], F32, tag="brk", name="brk")
        buq = cpool.tile([P, GH], F32, tag="buq", name="buq")
        brq = cpool.tile([P, GH // 2], F32, tag="brq", name="brq")
        buk = cpool.tile([P, GH], F32, tag="buk", name="buk")
        buv2 = cpool.tile([P, GH], F32, tag="buv2", name="buv2")
        # biases ride the scalar HWDGE queue: they are tiny, dep-free, and
        # software-DGE issue on Pool costs ~8us of Pool SEQ per DMA
        for name, tl in [("bdq", bdq), ("bdkv", bdkv), ("brk", brk),
                         ("buq", buq), ("brq", brq), ("buk", buk),
                         ("buv2", buv2)]:
            nc.scalar.dma_start(tl[:], t[name][:])

        C_QT = cqtp.tile([P, 12, QB], F8, tag="C_QT", name="C_QT")
        # fp8 queries for the DoubleRow score matmul: plane 0 = content Q,
        # plane 1 = rope Qr in its pair half, other half zeroed once here so
        # the unused lane pairs multiply against Kr as exact zeros
        Q8all = qtp.tile([P, 2, GH, QB], F8, tag="q8all", name="Q8all")
        nc.vector.memset(Q8all[:, 1, :, :], 0.0)

        # ---------- phase-C weight streams ----------
        def load_wuv(Gq):
            w = pcw.tile([P, 4, 512], BF16, tag="wuv", name=f"wuv{Gq}")
            ins = nc.sync.dma_start(w[:], t["wuv"][Gq])
            if Gq == 0:
                after_crit(ins)
            return w

        def load_wq(h):
            wuqh = pcw.tile([P, 12, DH], F8, tag="wuq", bufs=4, name=f"wuq{h}")
            i1 = nc.sync.dma_start(wuqh[:], t["wuq"][h])
            if h == 0:
                after_crit(i1)
            return wuqh

        def load_wrq(p):
            wrqp = pcw.tile([P, 12, 2 * DR], F8, tag="wrq", name=f"wrq{p}")
            i2 = nc.sync.dma_start(wrqp[:], t["wrq"][p])
            if p == 0:
                after_crit(i2)
            return wrqp

        def load_wuk(h):
            wukh = pcw.tile([P, 4, DH], F8, tag="wuk", name=f"wuk{h}")
            i3 = nc.sync.dma_start(wukh[:], t["wuk"][h])
            if h == 0:
                after_crit(i3)
            return wukh

        # ---------- Phase B (sharded) + AllGather ----------
        pbd = ctx.enter_context(tc.tile_pool(name="ph_b_dram", bufs=1,
                                             space="DRAM"))
        with tc.tile_pool(name="ph_b", bufs=1) as pb:
            # keep psg free of phase-B tiles: phase A's first psum allocs
            # would otherwise wait on the pack activations
            pack = pb.tile([P, 5, 256], BF16, tag="pack", name="pack")
            ps_m = [pss.tile([P, 256], F32, tag="s", name=f"psB_{m}")
                    for m in range(3)] + \
                   [pso.tile([P, 256], F32, tag="o", name="psB_3")]
            psk = psd.tile([DR, 256], F32, tag="d", name="psBk")
            for ch in range(4):
                for m in range(4):
                    for k8 in range(8):
                        ko = ch * 8 + k8
                        nc.tensor.matmul(ps_m[m][:],
                                         wdkv_chunks[ch][:, k8, m * P:(m + 1) * P],
                                         seqkb[:, ko, :],
                                         start=(ko == 0), stop=(ko == 31))
                for k8 in range(8):
                    ko = ch * 8 + k8
                    nc.tensor.matmul(psk[:], wrk[:, ko, :], seqkb[:, ko, :],
                                     start=(ko == 0), stop=(ko == 31))
            for m in range(4):
                nc.scalar.activation(pack[:, m, :], ps_m[m][:], AFT.Identity,
                                     bias=bdkv[:, m:m + 1])
            nc.scalar.activation(pack[0:DR, 4, :], psk[:], AFT.Identity,
                                 bias=brk[:, 0:1])
            gin = pbd.tile([P, 5, 256], BF16, tag="gin", name="gin")
            gout = pbd.tile([8, P, 5, 256], BF16, tag="gout", name="gout",
                            addr_space="Shared")
            # scalar's DMA queue drains early (only wdkv+wrk): the gather
            # input must not sit behind the multi-MB seq stream on sync
            gin_dma[0] = nc.scalar.dma_start(gin[:], pack[:]).ins
            nc.gpsimd.collective_compute(
                "AllGather",
                ALU.bypass,
                ins=[gin[:]],
                outs=[gout[:]],
                replica_groups=[list(range(8))],
            )

        # phase-C weight prefetches are emitted AFTER the collective so Pool's
        # SEQ reaches the collective instruction immediately (each SWDGE issue
        # costs ~8us of Pool SEQ time)
        wuv_next = load_wuv(0)
        wq_queue = [load_wq(0), load_wq(1), load_wq(2)]
        wrq_next = load_wrq(0)
        wuk_next = load_wuk(0)

        # ---------- Phase A: C_QT (streamed w_dq chunks of one m-tile) ----------
        for m in range(12):
            if m == 0:
                wq = wq0
            else:
                wq = pa.tile([P, 32, 128], F8, tag="wdqq", bufs=2,
                             name=f"wdqq{m}")
                last_wdqq[0] = nc.sync.dma_start(wq[:], t["wdq"][m]).ins
            ps = psg.tile([P, QB], F32, tag="psA", name=f"psA{m}")
            for ko in range(0, 32, 2):
                nc.tensor.matmul(ps[:], wq[:, ko:ko + 2, :],
                                 seqmy[:, ko:ko + 2, :],
                                 start=(ko == 0), stop=(ko == 30),
                                 perf_mode=DR_MODE)
            nc.scalar.activation(C_QT[:, m, :], ps[:], AFT.Identity,
                                 bias=bdq[:, m:m + 1])
        pa_cm.__exit__(None, None, None)

        iop_cm.__exit__(None, None, None)

        # ---------- Hoisted Q projections (overlap the AllGather) ----------
        # Qr for head pairs (2h, 2h+1) is packed into one 128-partition psum
        # so the D_R=64 matmuls keep the full PE array busy.
        for h in range(GH):
            wuqh = wq_queue.pop(0)
            if h + 3 < GH:
                wq_queue.append(load_wq(h + 3))
            ps = psg.tile([P, QB], F32, tag="psA", name=f"psQ{h}")
            for ko in range(0, 12, 2):
                nc.tensor.matmul(ps[:], wuqh[:, ko:ko + 2, :],
                                 C_QT[:, ko:ko + 2, :],
                                 start=(ko == 0), stop=(ko == 10),
                                 perf_mode=DR_MODE)
            nc.scalar.activation(Q8all[:, 0, h, :], ps[:], AFT.Identity,
                                 bias=buq[:, h:h + 1])
            if h % 2 == 1:
                p = h // 2
                wrqp = wrq_next
                if p < GH // 2 - 1:
                    wrq_next = load_wrq(p + 1)
                psr = psg.tile([P, QB], F32, tag="psA", name=f"psQr{p}")
                for ko in range(0, 12, 2):
                    nc.tensor.matmul(psr[:], wrqp[:, ko:ko + 2, :],
                                     C_QT[:, ko:ko + 2, :],
                                     start=(ko == 0), stop=(ko == 10),
                                     perf_mode=DR_MODE)
                nc.scalar.activation(Q8all[0:DR, 1, 2 * p, :], psr[0:DR, :],
                                     AFT.Identity, bias=brq[0:DR, p:p + 1])
                nc.scalar.activation(Q8all[DR:P, 1, 2 * p + 1, :],
                                     psr[DR:P, :], AFT.Identity,
                                     bias=brq[DR:P, p:p + 1])
        cqt_cm.__exit__(None, None, None)

        # ---------- unpack the AllGather result.  Emitted after the Q-loop
        # weight loads so this (long-waiting) DMA never head-of-line blocks
        # the wuq stream on the sync queue; kept OFF the Pool SWDGE queue
        # because SWDGE traffic concurrent with DoubleRow matmuls crashes the
        # PE exec unit.
        for m in range(4):
            nc.sync.dma_start(
                C_KVT[:, m, :].rearrange("p (r n) -> p r n", r=8),
                gout[:, :, m, :].rearrange("r p n -> p r n"))
        # Kr duplicated into partitions 64..127 for the odd pair halves
        for off in (0, DR):
            nc.sync.dma_start(
                KrT[off:off + DR, :].rearrange("p (r n) -> p r n", r=8),
                gout[:, 0:DR, 4, :].rearrange("r p n -> p r n"))
        # fp8 copy of the latent for the K-side DoubleRow projections (the
        # V side keeps the bf16 original: fp8 noise on V would land on the
        # output unattenuated)
        for m in range(4):
            nc.gpsimd.tensor_copy(C_KVT8[:, m, :], C_KVT[:, m, :])

        # ---------- Phase D pool + first weight pair (prefetch during C) ----------
        pd = ctx.enter_context(tc.tile_pool(name="ph_d", bufs=1))
        wout_tiles = []
        for half in range(2):
            w = pd.tile([P, 8, 512], BF16, tag="wout", bufs=4,
                        name=f"wout0_{half}")
            nc.sync.dma_start(w[:], t["wout"][0, half])
            wout_tiles.append(w)

        # ---------- Phase C: attention per head ----------
        with tc.tile_pool(name="ph_c", bufs=1) as pc:
            pending_den = []

            def flush_den(pd_item):
                hprev, psO_prev, s4a_prev, s4b_prev = pd_item
                psD_prev = psd.tile([P, QB], F32, tag="d", name=f"psD{hprev}")
                nc.tensor.matmul(psD_prev[:], ones128[:], s4a_prev[:],
                                 start=True, stop=False)
                nc.tensor.matmul(psD_prev[:], ones128[:], s4b_prev[:],
                                 start=False, stop=True)
                recip = pc.tile([P, QB], F32, tag="recip", bufs=1,
                                name=f"recip{hprev}")
                nc.vector.reciprocal(recip[:], psD_prev[:])
                nc.vector.tensor_tensor(OT[:, hprev, :], psO_prev[:], recip[:],
                                        ALU.mult)
                nc.scalar.activation(OT[:, hprev, :], OT[:, hprev, :], AFT.Identity,
                                     bias=buv2[:, hprev:hprev + 1])

            for Gq in range(4):
                wuvG = wuv_next
                V_G = pc.tile([P, 16, 512], BF16, tag="vg", bufs=2, name=f"vg{Gq}")
                for kt in range(16):
                    ps = psg.tile([P, 512], F32, tag="psA", name=f"psVg{Gq}_{kt}")
                    for ci in range(4):
                        nc.tensor.matmul(ps[:], C_KVT[:, ci, kt * P:(kt + 1) * P],
                                         wuvG[:, ci, :],
                                         start=(ci == 0), stop=(ci == 3))
                    # copy on ACT: Pool cannot read PSUM, DVE is loaded with
                    # the den reduces
                    nc.scalar.activation(V_G[:, kt, :], ps[:], AFT.Copy)
                if Gq < 3:
                    wuv_next = load_wuv(Gq + 1)
                # fp8 keys for the whole 4-head group: planes 0-3 = content K
                # per head, plane 4 = rope Kr (shared; already duplicated into
                # both partition halves).  The DoubleRow lhsT for head h4
                # picks planes {h4, 4} via a strided view.
                KALL = pc.tile([P, 5, S], F8, tag="ktile", bufs=2,
                               name=f"ktile{Gq}")
                nc.gpsimd.tensor_copy(KALL[:, 4, :], KrT[:])
                for h4 in range(4):
                    h = Gq * 4 + h4
                    wukh = wuk_next
                    if h < GH - 1:
                        wuk_next = load_wuk(h + 1)

                    for kb in range(4):
                        psk = psg.tile([P, 512], F32, tag="psA", name=f"psKT{h}_{kb}")
                        for ci in range(0, 4, 2):
                            nc.tensor.matmul(psk[:], wukh[:, ci:ci + 2, :],
                                             C_KVT8[:, ci:ci + 2,
                                                    kb * 512:(kb + 1) * 512],
                                             start=(ci == 0), stop=(ci == 2),
                                             perf_mode=DR_MODE)
                        nc.scalar.activation(KALL[:, h4, kb * 512:(kb + 1) * 512],
                                             psk[:], AFT.Identity,
                                             bias=buk[:, h:h + 1])

                    PT = pc.tile([P, 16, QB], BF16, tag="pt", bufs=2, name=f"pt{h}")
                    psO = pso.tile([P, QB], F32, tag="o", name=f"psO{h}")
                    # denominator halves: DVE reduces 8 kt-tiles per partition
                    # (f32 accumulate) as soon as their exps land; the
                    # cross-partition ones-matmuls are deferred a full head
                    # (flush_den) so the PE never waits on the DVE chain
                    s4 = {}

                    def den_half(half):
                        s4f = pc.tile([P, QB], F32, tag="ds4f", bufs=1,
                                      name=f"ds4f_{h}_{half}")
                        nc.vector.tensor_reduce(
                            s4f[:],
                            PT[:, 8 * half:8 * half + 8, :]
                            .rearrange("p k q -> p q k"),
                            mybir.AxisListType.X, ALU.add)
                        s4[half] = pc.tile([P, QB], BF16, tag="ds4", bufs=4,
                                           name=f"ds4_{h}_{half}")
                        nc.vector.tensor_copy(s4[half][:], s4f[:])

                    pending = None
                    for kt in range(16):
                        psS = pss.tile([P, QB], F32, tag="s", name=f"psS{h}_{kt}")
                        nc.tensor.matmul(psS[:],
                                         KALL[:, h4::4 - h4,
                                              kt * P:(kt + 1) * P],
                                         Q8all[:, :, h, :],
                                         start=True, stop=True,
                                         perf_mode=DR_MODE)
                        nc.scalar.activation(PT[:, kt, :], psS[:], AFT.Exp,
                                             scale=SCALER)
                        if kt == 8:
                            den_half(0)
                        if pending is not None:
                            kp = pending
                            nc.tensor.matmul(psO[:], V_G[:, kp, h4 * P:(h4 + 1) * P],
                                             PT[:, kp, :],
                                             start=(kp == 0), stop=False)
                        pending = kt
                    kp = pending
                    nc.tensor.matmul(psO[:], V_G[:, kp, h4 * P:(h4 + 1) * P],
                                     PT[:, kp, :], start=False, stop=True)
                    den_half(1)

                    if pending_den:
                        flush_den(pending_den.pop(0))
                    pending_den.append((h, psO, s4[0], s4[1]))
            for item in pending_den:
                flush_den(item)

        # ---------- Phase D: partial out-projection ----------
        for nt in range(8):
            if nt == 0:
                wha, whb = wout_tiles
            else:
                wha = pd.tile([P, 8, 512], BF16, tag="wout", bufs=4,
                              name=f"wouta{nt}")
                nc.sync.dma_start(wha[:], t["wout"][nt, 0])
                whb = pd.tile([P, 8, 512], BF16, tag="wout", bufs=4,
                              name=f"woutb{nt}")
                nc.sync.dma_start(whb[:], t["wout"][nt, 1])
            for qt in range(4):
                ps = psg.tile([P, 512], F32, tag="psA", name=f"psOut{nt}_{qt}")
                for hh in range(GH):
                    w = wha if hh < 8 else whb
                    nc.tensor.matmul(ps[:], OT[:, hh, qt * P:(qt + 1) * P],
                                     w[:, hh % 8, :],
                                     start=(hh == 0), stop=(hh == GH - 1))
                osb = pd.tile([P, 512], F16, tag="osb", bufs=3,
                              name=f"osb{nt}_{qt}")
                nc.scalar.activation(osb[:], ps[:], AFT.Copy)
                nc.sync.dma_start(
                    t["pout"][qt * P:(qt + 1) * P, nt * 512:(nt + 1) * 512],
                    osb[:])


def _build_program(rep=1):
    nc = bacc.Bacc("TRN2", target_bir_lowering=False, debug=False)
    t = {}
    t["seqT_my"] = nc.dram_tensor("t_seqT_my", [P, 32, QB], F8, kind="ExternalInput")
    t["seqT_mykb"] = nc.dram_tensor("t_seqT_mykb", [P, 32, 256], BF16, kind="ExternalInput")
    t["wdq"] = nc.dram_tensor("t_wdq", [12, P, 32, 128], F8, kind="ExternalInput")
    t["wdkv"] = nc.dram_tensor("t_wdkv", [P, 32, DC], BF16, kind="ExternalInput")
    t["wrk"] = nc.dram_tensor("t_wrk", [P, 32, DR], BF16, kind="ExternalInput")
    t["wuq"] = nc.dram_tensor("t_wuq", [GH, P, 12, DH], F8, kind="ExternalInput")
    t["wrq"] = nc.dram_tensor("t_wrq", [GH // 2, P, 12, 2 * DR], F8, kind="ExternalInput")
    t["wuk"] = nc.dram_tensor("t_wuk", [GH, P, 4, DH], F8, kind="ExternalInput")
    t["wuv"] = nc.dram_tensor("t_wuv", [4, P, 4, 512], BF16, kind="ExternalInput")
    t["wout"] = nc.dram_tensor("t_wout", [8, 2, P, 8, 512], BF16, kind="ExternalInput")
    t["bdq"] = nc.dram_tensor("t_bdq", [P, 12], F32, kind="ExternalInput")
    t["bdkv"] = nc.dram_tensor("t_bdkv", [P, 4], F32, kind="ExternalInput")
    t["brk"] = nc.dram_tensor("t_brk", [DR, 1], F32, kind="ExternalInput")
    t["buq"] = nc.dram_tensor("t_buq", [P, GH], F32, kind="ExternalInput")
    t["brq"] = nc.dram_tensor("t_brq", [P, GH // 2], F32, kind="ExternalInput")
    t["buk"] = nc.dram_tensor("t_buk", [P, GH], F32, kind="ExternalInput")
    t["buv2"] = nc.dram_tensor("t_buv2", [P, GH], F32, kind="ExternalInput")
    t["pout"] = nc.dram_tensor("t_pout", [QB, DM], F16, kind="ExternalOutput")

    with tile.TileContext(nc) as tc:
        for _ in range(rep):
            _emit_body(nc, tc, t)
    nc.compile()
    return nc


def _prep_shared(inputs):
    """Host-side layout + bf16 cast. Returns dict of shared arrays and
    per-head-group arrays."""
    bf = ml_dtypes.bfloat16
    f8 = ml_dtypes.float8_e4m3
    f32 = np.float32
    seq = np.asarray(inputs["sequence"], dtype=np.float32)[0]      # [2048, 4096]
    w_dq = np.asarray(inputs["w_dq"], dtype=np.float32)
    b_dq = np.asarray(inputs["b_dq"], dtype=np.float32)
    w_dkv = np.asarray(inputs["w_dkv"], dtype=np.float32)
    b_dkv = np.asarray(inputs["b_dkv"], dtype=np.float32)
    w_rk = np.asarray(inputs["w_rk"], dtype=np.float32)
    b_rk = np.asarray(inputs["b_rk"], dtype=np.float32)
    w_uq = np.asarray(inputs["w_uq"], dtype=np.float32)
    b_uq = np.asarray(inputs["b_uq"], dtype=np.float32)
    w_rq = np.asarray(inputs["w_rq"], dtype=np.float32)
    b_rq = np.asarray(inputs["b_rq"], dtype=np.float32)
    w_uk = np.asarray(inputs["w_uk"], dtype=np.float32)
    b_uk = np.asarray(inputs["b_uk"], dtype=np.float32)
    w_uv = np.asarray(inputs["w_uv"], dtype=np.float32)
    b_uv = np.asarray(inputs["b_uv"], dtype=np.float32)
    w_out = np.asarray(inputs["w_out"], dtype=np.float32)

    shared = {
        "seqT4": seq.reshape(4, 512, 32, P).transpose(0, 3, 2, 1).astype(f8),
        "seqT4b": seq.reshape(4, 512, 32, P).transpose(0, 3, 2, 1).astype(bf),
        "wdq": w_dq.reshape(32, P, 12, 128).transpose(2, 1, 0, 3).astype(f8),
        "wdkv": w_dkv.reshape(32, P, DC).transpose(1, 0, 2).astype(bf),
        "wrk": w_rk.reshape(32, P, DR).transpose(1, 0, 2).astype(bf),
        "bdq": np.ascontiguousarray(b_dq.reshape(12, P).T, dtype=f32),
        "bdkv": np.ascontiguousarray(b_dkv.reshape(4, P).T, dtype=f32),
        "brk": np.ascontiguousarray(b_rk.reshape(DR, 1), dtype=f32),
    }
    per_g = []
    for gi in range(NG):
        cols = slice(gi * GH * DH, (gi + 1) * GH * DH)       # 2048 cols
        c1k = slice(gi * GH * DR, (gi + 1) * GH * DR)        # 1024 cols
        per_g.append({
            "wuq": w_uq[:, cols].reshape(12, P, GH, DH).transpose(2, 1, 0, 3).astype(f8),
            "wrq": w_rq[:, c1k].reshape(12, P, GH // 2, 2 * DR).transpose(2, 1, 0, 3).astype(f8),
            "wuk": w_uk[:, cols].reshape(4, P, GH, DH).transpose(2, 1, 0, 3).astype(f8),
            "wuv": w_uv[:, cols].reshape(4, P, 4, 512).transpose(2, 1, 0, 3).astype(bf),
            "wout": w_out[cols, :].reshape(2, 8, P, 8, 512).transpose(3, 0, 2, 1, 4).astype(bf),
            "buv2": np.ascontiguousarray(b_uv[cols].reshape(GH, P).T, dtype=f32),
            "buq": np.ascontiguousarray(b_uq[cols].reshape(GH, P).T, dtype=f32),
            "brq": np.ascontiguousarray(b_rq[c1k].reshape(GH // 2, 2 * DR).T, dtype=f32),
            "buk": np.ascontiguousarray(b_uk[cols].reshape(GH, P).T, dtype=f32),
        })
    return shared, per_g


def _prep_in_maps(inputs):
    shared, per_g = _prep_shared(inputs)
    in_maps = []
    for c in range(8):
        gi, ri = c % NG, c // NG
        m = dict(shared)
        m.update(per_g[gi])
        m["seqT_my"] = np.ascontiguousarray(shared["seqT4"][ri])
        kb, half = c // 2, c % 2
        m["seqT_mykb"] = np.ascontiguousarray(
            shared["seqT4b"][kb][:, :, half * 256:(half + 1) * 256])
        del m["seqT4"]
        del m["seqT4b"]
        in_maps.append({f"t_{k}": v for k, v in m.items()})
    return in_maps


class _Runner:
    """Cached sharded PJRT executor for a compiled Bass program."""

    def __init__(self, nc):
        bass2jax.install_neuronx_cc_hook()
        self.nc = nc
        in_names, out_names, out_avals = [], [], []
        pid_name = nc.partition_id_tensor.name if nc.partition_id_tensor else None
        for alloc in nc.m.functions[0].allocations:
            if not isinstance(alloc, mybir.MemoryLocationSet):
                continue
            name = alloc.memorylocations[0].name
            if alloc.kind == "ExternalInput":
                if name != pid_name:
                    in_names.append(name)
            elif alloc.kind == "ExternalOutput":
                out_names.append(name)
                shape = tuple(alloc.tensor_shape)
                dtype = mybir.dt.np(alloc.dtype)
                out_avals.append(jax.core.ShapedArray(shape, dtype))
        self.in_names = in_names
        self.out_names = out_names
        all_in_names = list(in_names) + list(out_names)
        if pid_name is not None:
            all_in_names.append(pid_name)

        def _body(*args):
            operands = list(args)
            if nc.partition_id_tensor is not None:
                operands.append(bass2jax.partition_id_tensor())
            outs = bass2jax._bass_exec_p.bind(
                *operands,
                out_avals=tuple(out_avals),
                in_names=tuple(all_in_names),
                out_names=tuple(out_names),
                lowering_input_output_aliases=(),
                sim_require_finite=True,
                sim_require_nnan=True,
                nc=nc,
            )
            return tuple(outs)

        devices = jax.devices()[:8]
        self.mesh = Mesh(np.asarray(devices), ("core",))
        n_io = len(in_names) + len(out_names)
        self.fn = jax.jit(
            shard_map(_body, mesh=self.mesh,
                      in_specs=(PartitionSpec("core"),) * n_io,
                      out_specs=(PartitionSpec("core"),) * len(out_names),
                      check_rep=False),
            keep_unused=True)
        self.sharding = NamedSharding(self.mesh, PartitionSpec("core"))
        self.dev_zero = [
            jax.device_put(
                np.zeros((8 * av.shape[0], *av.shape[1:]), av.dtype), self.sharding)
            for av in out_avals]
        self.out_avals = out_avals

    def stage(self, in_maps):
        dev_in = []
        for name in self.in_names:
            cat = np.concatenate([np.asarray(m[name]) for m in in_maps], axis=0)
            dev_in.append(jax.device_put(cat, self.sharding))
        return dev_in

    def run_staged(self, dev_in):
        outs = self.fn(*dev_in, *self.dev_zero)
        jax.block_until_ready(outs)
        return outs

    def run(self, in_maps):
        outs = self.run_staged(self.stage(in_maps))
        res = []
        for c in range(8):
            d = {}
            for i, name in enumerate(self.out_names):
                av = self.out_avals[i]
                d[name] = np.asarray(outs[i]).reshape(8, *av.shape)[c]
            res.append(d)
        return res


_CTX = None


def _get_ctx():
    global _CTX
    if _CTX is None:
        nc = _build_program(rep=1)
        _CTX = _Runner(nc)
    return _CTX


def kernel(**inputs):
    runner = _get_ctx()
    in_maps = _prep_in_maps(inputs)
    res = runner.run(in_maps)
    b_out = np.asarray(inputs["b_out"], dtype=np.float32)
    out = np.empty((S, DM), dtype=np.float32)
    for ri in range(NR):
        acc = res[ri * NG + 0]["t_pout"].astype(np.float32)
        for gi in range(1, NG):
            acc += res[ri * NG + gi]["t_pout"].astype(np.float32)
        out[ri * QB:(ri + 1) * QB] = acc + b_out
    return out.reshape(1, S, DM)



# revision 42
# speedup vs baseline: 2.4056x; 2.4056x over previous
"""MLA (multi-head latent attention) Trainium2 kernel, 8-core SPMD.

Sharding: 2 head-groups x 4 query-row-groups grid over 8 NeuronCores.
  core c: gi = c % 2  -> heads [gi*16, gi*16+16)  (of H=32)
          ri = c // 2 -> query rows [ri*512, ri*512+512)  (of S=2048)
Each core computes a partial output  pout = O(its heads, its rows) @ w_out[rows of its heads]
in fp16; the host sums the two head-group partials per row block and adds b_out.

Precision: matmuls on error-attenuated paths (C_Q, Q/Qr projections, the
K-side latent, and the Q.K^T scores) run in fp8e4m3 with DoubleRow perf
mode (2 contraction tiles per PE pass); the V path, att.V, and the output
projection stay bf16 with fp32 PSUM so fp8 noise never reaches the output
unattenuated (measured end-to-end L2 rel err ~6e-3 vs the fp32 reference).
The softmax denominator is computed as a per-partition DVE tensor_reduce
over the exp tiles plus one 128-wide ones-matmul per head.  Softmax skips
max-subtraction: logits are bounded for this problem's scale.

NOTE: SWDGE (Pool-queue) DMA traffic concurrent with DoubleRow matmuls
crashes the PE exec unit; all streams ride the SP/ACT hardware DGE queues
and the Pool queue carries only the AllGather.

Self-contained: shapes/layouts hardcoded; host does layout/cast/shard,
device kernel does all matmul/softmax work, host sums 2 partials per row block.
"""

import numpy as np
import ml_dtypes

import jax
from jax.sharding import Mesh, PartitionSpec, NamedSharding
try:
    from jax.experimental.shard_map import shard_map
except ImportError:  # newer jax
    from jax import shard_map

import concourse.tile as tile
from concourse import bacc, mybir
from concourse import bass2jax

BF16 = mybir.dt.bfloat16
F32 = mybir.dt.float32
F16 = mybir.dt.float16
F8 = mybir.dt.float8e4
DR_MODE = mybir.MatmulPerfMode.DoubleRow
AFT = mybir.ActivationFunctionType
ALU = mybir.AluOpType

# problem dims
S, DE, DC1, DC, DR, H, DH, DM = 2048, 4096, 1536, 512, 64, 32, 128, 4096
NG, NR = 2, 4           # head groups x row groups = 8 cores
GH = H // NG            # 16 heads per core
QB = S // NR            # 512 query rows per core
SCALER = float(1.0 / np.sqrt(np.float32(DH + DR)))
P = 128


def _emit_body(nc, tc, t):
    """Emit one full iteration of the per-core computation.

    Phase order (chosen so the AllGather hides under independent PE work):
      B-shard: C_KVT/KrT for this core's 256-key slice  -> AllGather kickoff
      A:       C_QT (full, this core's 512 query rows)
      QT-all:  Q/Qr projections for all 16 heads (needs only C_QT)
      C:       per head: KT, scores^T, exp, AV, denominators
      D:       partial out-projection
    """
    from contextlib import ExitStack
    from concourse.tile_rust import add_dep_helper

    with ExitStack() as ctx:
        # PSUM pools: 3+2+1+2 = 8 banks exactly
        psg = ctx.enter_context(tc.tile_pool(name="psg", bufs=2, space="PSUM"))
        pss = ctx.enter_context(tc.tile_pool(name="pss", bufs=3, space="PSUM"))
        psd = ctx.enter_context(tc.tile_pool(name="psd", bufs=1, space="PSUM"))
        pso = ctx.enter_context(tc.tile_pool(name="pso", bufs=2, space="PSUM"))

        cpool = ctx.enter_context(tc.tile_pool(name="persist", bufs=1))
        pcw = ctx.enter_context(tc.tile_pool(name="pcw", bufs=2))
        qtp = ctx.enter_context(tc.tile_pool(name="qtp", bufs=1))
        cqt_cm = tc.tile_pool(name="cqt", bufs=1)
        cqtp = cqt_cm.__enter__()
        iop_cm = tc.tile_pool(name="iop", bufs=1)
        iop = iop_cm.__enter__()
        pa_cm = tc.tile_pool(name="ph_a", bufs=1)
        pa = pa_cm.__enter__()

        # ---------- DMA ordering helpers ----------
        gin_dma = [None]
        crit_dmas = []

        def after_crit(bass_inst, n=None):
            for d in (crit_dmas if n is None else crit_dmas[:n]):
                add_dep_helper(bass_inst.ins, d,
                               reason="defer until B-critical DMAs issued")
            return bass_inst

        def after_gin(bass_inst):
            if gin_dma[0] is not None:
                add_dep_helper(bass_inst.ins, gin_dma[0],
                               reason="defer until collective input sent")
            return bass_inst

        # ---------- B-critical loads ----------
        wdkv_chunks = []
        for ch in range(4):
            wch = iop.tile([P, 8, DC], BF16, tag="wdkv", bufs=2, name=f"wdkv{ch}")
            if ch == 0:
                for hh in range(2):
                    ins = nc.scalar.dma_start(
                        wch[:, hh * 4:(hh + 1) * 4, :],
                        t["wdkv"][:, hh * 4:(hh + 1) * 4, :])
                    crit_dmas.append(ins.ins)
            else:
                ins = nc.scalar.dma_start(wch[:],
                                          t["wdkv"][:, ch * 8:(ch + 1) * 8, :])
                crit_dmas.append(ins.ins)
            wdkv_chunks.append(wch)
        wrk = iop.tile([P, 32, DR], BF16, tag="wrk", name="wrk")
        crit_dmas.append(nc.scalar.dma_start(wrk[:], t["wrk"][:]).ins)
        # seqkb chunk 0 first, then wq0 (phase A's first weight) slotted in
        # before the rest so phase A can start the moment phase B's PE work
        # drains; seqmy chunks stream behind and are consumed in ko order
        seqkb = iop.tile([P, 32, 256], BF16, tag="seqkb", name="seqkb")
        wq0 = pa.tile([P, 32, 128], F8, tag="wdqq", bufs=2, name="wdqq0")
        seqmy = pa.tile([P, 32, QB], F8, tag="seqmy", name="seqmy")
        ins = nc.sync.dma_start(seqkb[:, 0:8, :], t["seqT_mykb"][:, 0:8, :])
        crit_dmas.append(ins.ins)
        nc.sync.dma_start(wq0[:], t["wdq"][0])
        for ch in range(1, 4):
            ins = nc.sync.dma_start(seqkb[:, ch * 8:(ch + 1) * 8, :],
                                    t["seqT_mykb"][:, ch * 8:(ch + 1) * 8, :])
            crit_dmas.append(ins.ins)
        for ch in range(8):
            nc.sync.dma_start(seqmy[:, ch * 4:(ch + 1) * 4, :],
                              t["seqT_my"][:, ch * 4:(ch + 1) * 4, :])
        last_wdqq = [None]

        # ---------- persistent tiles ----------
        C_KVT = cpool.tile([P, 4, S], BF16, tag="C_KVT", name="C_KVT")
        C_KVT8 = cpool.tile([P, 4, S], F8, tag="C_KVT8", name="C_KVT8")
        KrT = cpool.tile([P, S], BF16, tag="KrT", name="KrT")
        OTa = cpool.tile([P, GH // 2, QB], BF16, tag="OTa", name="OTa")
        OTb = cpool.tile([P, GH // 2, QB], BF16, tag="OTb", name="OTb")

        def OT(hh):
            return (OTa if hh < GH // 2 else OTb)[:, hh % (GH // 2), :]
        ones128 = cpool.tile([P, P], BF16, tag="ones128", name="ones128")
        nc.any.memset(ones128[:], 1.0)
        bdq = cpool.tile([P, 12], F32, tag="bdq", name="bdq")
        bdkv = cpool.tile([P, 4], F32, tag="bdkv", name="bdkv")
        brk = cpool.tile([DR, 1], F32, tag="brk", name="brk")
        buq = cpool.tile([P, GH], F32, tag="buq", name="buq")
        brq = cpool.tile([P, GH // 2], F32, tag="brq", name="brq")
        buk = cpool.tile([P, GH], F32, tag="buk", name="buk")
        buv2 = cpool.tile([P, GH], F32, tag="buv2", name="buv2")
        # biases ride the scalar HWDGE queue: they are tiny, dep-free, and
        # software-DGE issue on Pool costs ~8us of Pool SEQ per DMA
        for name, tl in [("bdq", bdq), ("bdkv", bdkv), ("brk", brk),
                         ("buq", buq), ("brq", brq), ("buk", buk),
                         ("buv2", buv2)]:
            nc.scalar.dma_start(tl[:], t[name][:])

        C_QT = cqtp.tile([P, 12, QB], F8, tag="C_QT", name="C_QT")
        # fp8 queries for the DoubleRow score matmul: plane 0 = content Q,
        # plane 1 = rope Qr in its pair half, other half zeroed once here so
        # the unused lane pairs multiply against Kr as exact zeros
        Q8all = qtp.tile([P, 2, GH, QB], F8, tag="q8all", name="Q8all")
        nc.vector.memset(Q8all[:, 1, :, :], 0.0)

        # ---------- phase-C weight streams ----------
        def load_wuv(Gq):
            w = pcw.tile([P, 4, 512], BF16, tag="wuv", name=f"wuv{Gq}")
            ins = nc.sync.dma_start(w[:], t["wuv"][Gq])
            if Gq == 0:
                after_crit(ins)
            return w

        def load_wq(h):
            wuqh = pcw.tile([P, 12, DH], F8, tag="wuq", bufs=4, name=f"wuq{h}")
            i1 = nc.sync.dma_start(wuqh[:], t["wuq"][h])
            if h == 0:
                after_crit(i1)
            return wuqh

        def load_wrq(p):
            wrqp = pcw.tile([P, 12, 2 * DR], F8, tag="wrq", name=f"wrq{p}")
            i2 = nc.sync.dma_start(wrqp[:], t["wrq"][p])
            if p == 0:
                after_crit(i2)
            return wrqp

        def load_wuk(h):
            wukh = pcw.tile([P, 4, DH], F8, tag="wuk", name=f"wuk{h}")
            i3 = nc.sync.dma_start(wukh[:], t["wuk"][h])
            if h == 0:
                after_crit(i3)
            return wukh

        # ---------- Phase B (sharded) + AllGather ----------
        pbd = ctx.enter_context(tc.tile_pool(name="ph_b_dram", bufs=1,
                                             space="DRAM"))
        with tc.tile_pool(name="ph_b", bufs=1) as pb:
            # keep psg free of phase-B tiles: phase A's first psum allocs
            # would otherwise wait on the pack activations
            pack = pb.tile([P, 5, 256], BF16, tag="pack", name="pack")
            ps_m = [pss.tile([P, 256], F32, tag="s", name=f"psB_{m}")
                    for m in range(3)] + \
                   [pso.tile([P, 256], F32, tag="o", name="psB_3")]
            psk = psd.tile([DR, 256], F32, tag="d", name="psBk")
            for ch in range(4):
                for m in range(4):
                    for k8 in range(8):
                        ko = ch * 8 + k8
                        nc.tensor.matmul(ps_m[m][:],
                                         wdkv_chunks[ch][:, k8, m * P:(m + 1) * P],
                                         seqkb[:, ko, :],
                                         start=(ko == 0), stop=(ko == 31))
                for k8 in range(8):
                    ko = ch * 8 + k8
                    nc.tensor.matmul(psk[:], wrk[:, ko, :], seqkb[:, ko, :],
                                     start=(ko == 0), stop=(ko == 31))
            for m in range(4):
                nc.scalar.activation(pack[:, m, :], ps_m[m][:], AFT.Identity,
                                     bias=bdkv[:, m:m + 1])
            nc.scalar.activation(pack[0:DR, 4, :], psk[:], AFT.Identity,
                                 bias=brk[:, 0:1])
            gin = pbd.tile([P, 5, 256], BF16, tag="gin", name="gin")
            gout = pbd.tile([8, P, 5, 256], BF16, tag="gout", name="gout",
                            addr_space="Shared")
            # scalar's DMA queue drains early (only wdkv+wrk): the gather
            # input must not sit behind the multi-MB seq stream on sync
            gin_dma[0] = nc.scalar.dma_start(gin[:], pack[:]).ins
            nc.gpsimd.collective_compute(
                "AllGather",
                ALU.bypass,
                ins=[gin[:]],
                outs=[gout[:]],
                replica_groups=[list(range(8))],
            )

        # phase-C weight prefetches are emitted AFTER the collective so Pool's
        # SEQ reaches the collective instruction immediately (each SWDGE issue
        # costs ~8us of Pool SEQ time)
        wuv_next = load_wuv(0)
        wq_queue = [load_wq(0), load_wq(1), load_wq(2)]
        wrq_next = load_wrq(0)
        wuk_next = load_wuk(0)

        # ---------- Phase A: C_QT (streamed w_dq chunks of one m-tile) ----------
        for m in range(12):
            if m == 0:
                wq = wq0
            else:
                wq = pa.tile([P, 32, 128], F8, tag="wdqq", bufs=2,
                             name=f"wdqq{m}")
                last_wdqq[0] = nc.sync.dma_start(wq[:], t["wdq"][m]).ins
            ps = psg.tile([P, QB], F32, tag="psA", name=f"psA{m}")
            for ko in range(0, 32, 2):
                nc.tensor.matmul(ps[:], wq[:, ko:ko + 2, :],
                                 seqmy[:, ko:ko + 2, :],
                                 start=(ko == 0), stop=(ko == 30),
                                 perf_mode=DR_MODE)
            nc.scalar.activation(C_QT[:, m, :], ps[:], AFT.Identity,
                                 bias=bdq[:, m:m + 1])
        pa_cm.__exit__(None, None, None)

        iop_cm.__exit__(None, None, None)

        # ---------- Hoisted Q projections (overlap the AllGather) ----------
        # Qr for head pairs (2h, 2h+1) is packed into one 128-partition psum
        # so the D_R=64 matmuls keep the full PE array busy.
        for h in range(GH):
            wuqh = wq_queue.pop(0)
            if h + 3 < GH:
                wq_queue.append(load_wq(h + 3))
            ps = psg.tile([P, QB], F32, tag="psA", name=f"psQ{h}")
            for ko in range(0, 12, 2):
                nc.tensor.matmul(ps[:], wuqh[:, ko:ko + 2, :],
                                 C_QT[:, ko:ko + 2, :],
                                 start=(ko == 0), stop=(ko == 10),
                                 perf_mode=DR_MODE)
            nc.scalar.activation(Q8all[:, 0, h, :], ps[:], AFT.Identity,
                                 bias=buq[:, h:h + 1])
            if h % 2 == 1:
                p = h // 2
                wrqp = wrq_next
                if p < GH // 2 - 1:
                    wrq_next = load_wrq(p + 1)
                psr = psg.tile([P, QB], F32, tag="psA", name=f"psQr{p}")
                for ko in range(0, 12, 2):
                    nc.tensor.matmul(psr[:], wrqp[:, ko:ko + 2, :],
                                     C_QT[:, ko:ko + 2, :],
                                     start=(ko == 0), stop=(ko == 10),
                                     perf_mode=DR_MODE)
                nc.scalar.activation(Q8all[0:DR, 1, 2 * p, :], psr[0:DR, :],
                                     AFT.Identity, bias=brq[0:DR, p:p + 1])
                nc.scalar.activation(Q8all[DR:P, 1, 2 * p + 1, :],
                                     psr[DR:P, :], AFT.Identity,
                                     bias=brq[DR:P, p:p + 1])
        cqt_cm.__exit__(None, None, None)

        # ---------- unpack the AllGather result.  Emitted after the Q-loop
        # weight loads so this (long-waiting) DMA never head-of-line blocks
        # the wuq stream on the sync queue; kept OFF the Pool SWDGE queue
        # because SWDGE traffic concurrent with DoubleRow matmuls crashes the
        # PE exec unit.
        for m in range(4):
            nc.sync.dma_start(
                C_KVT[:, m, :].rearrange("p (r n) -> p r n", r=8),
                gout[:, :, m, :].rearrange("r p n -> p r n"))
        # Kr duplicated into partitions 64..127 for the odd pair halves
        for off in (0, DR):
            nc.sync.dma_start(
                KrT[off:off + DR, :].rearrange("p (r n) -> p r n", r=8),
                gout[:, 0:DR, 4, :].rearrange("r p n -> p r n"))
        # fp8 copy of the latent for the K-side DoubleRow projections (the
        # V side keeps the bf16 original: fp8 noise on V would land on the
        # output unattenuated)
        for m in range(4):
            nc.gpsimd.tensor_copy(C_KVT8[:, m, :], C_KVT[:, m, :])

        # ---------- Phase D pool + first weight pair (prefetch during C) ----------
        pd = ctx.enter_context(tc.tile_pool(name="ph_d", bufs=1))
        wout_tiles = []
        for half in range(2):
            w = pd.tile([P, 8, 512], BF16, tag="wout", bufs=4,
                        name=f"wout0_{half}")
            nc.sync.dma_start(w[:], t["wout"][0, half])
            wout_tiles.append(w)

        # ---------- Phase C: attention per head ----------
        with tc.tile_pool(name="ph_c", bufs=1) as pc:
            pending_den = []

            def flush_den(pd_item):
                hprev, psO_prev, s4a_prev, s4b_prev = pd_item
                psD_prev = psd.tile([P, QB], F32, tag="d", name=f"psD{hprev}")
                nc.tensor.matmul(psD_prev[:], ones128[:], s4a_prev[:],
                                 start=True, stop=False)
                nc.tensor.matmul(psD_prev[:], ones128[:], s4b_prev[:],
                                 start=False, stop=True)
                recip = pc.tile([P, QB], F32, tag="recip", bufs=1,
                                name=f"recip{hprev}")
                nc.vector.reciprocal(recip[:], psD_prev[:])
                nc.vector.tensor_tensor(OT(hprev), psO_prev[:], recip[:],
                                        ALU.mult)
                nc.scalar.activation(OT(hprev), OT(hprev), AFT.Identity,
                                     bias=buv2[:, hprev:hprev + 1])

            for Gq in range(4):
                wuvG = wuv_next
                V_G = pc.tile([P, 16, 512], BF16, tag="vg", bufs=2, name=f"vg{Gq}")
                for kt in range(16):
                    ps = psg.tile([P, 512], F32, tag="psA", name=f"psVg{Gq}_{kt}")
                    for ci in range(4):
                        nc.tensor.matmul(ps[:], C_KVT[:, ci, kt * P:(kt + 1) * P],
                                         wuvG[:, ci, :],
                                         start=(ci == 0), stop=(ci == 3))
                    # copy on ACT: Pool cannot read PSUM, DVE is loaded with
                    # the den reduces
                    nc.scalar.activation(V_G[:, kt, :], ps[:], AFT.Copy)
                if Gq < 3:
                    wuv_next = load_wuv(Gq + 1)
                # fp8 keys for the whole 4-head group: planes 0-3 = content K
                # per head, plane 4 = rope Kr (shared; already duplicated into
                # both partition halves).  The DoubleRow lhsT for head h4
                # picks planes {h4, 4} via a strided view.
                KALL = pc.tile([P, 5, S], F8, tag="ktile", bufs=2,
                               name=f"ktile{Gq}")
                nc.gpsimd.tensor_copy(KALL[:, 4, :], KrT[:])
                for h4 in range(4):
                    h = Gq * 4 + h4
                    wukh = wuk_next
                    if h < GH - 1:
                        wuk_next = load_wuk(h + 1)

                    for kb in range(4):
                        psk = psg.tile([P, 512], F32, tag="psA", name=f"psKT{h}_{kb}")
                        for ci in range(0, 4, 2):
                            nc.tensor.matmul(psk[:], wukh[:, ci:ci + 2, :],
                                             C_KVT8[:, ci:ci + 2,
                                                    kb * 512:(kb + 1) * 512],
                                             start=(ci == 0), stop=(ci == 2),
                                             perf_mode=DR_MODE)
                        nc.scalar.activation(KALL[:, h4, kb * 512:(kb + 1) * 512],
                                             psk[:], AFT.Identity,
                                             bias=buk[:, h:h + 1])

                    PT = pc.tile([P, 16, QB], BF16, tag="pt", bufs=2, name=f"pt{h}")
                    psO = pso.tile([P, QB], F32, tag="o", name=f"psO{h}")
                    # denominator halves: DVE reduces 8 kt-tiles per partition
                    # (f32 accumulate) as soon as their exps land; the
                    # cross-partition ones-matmuls are deferred a full head
                    # (flush_den) so the PE never waits on the DVE chain
                    s4 = {}

                    def den_half(half):
                        s4f = pc.tile([P, QB], F32, tag="ds4f", bufs=1,
                                      name=f"ds4f_{h}_{half}")
                        nc.vector.tensor_reduce(
                            s4f[:],
                            PT[:, 8 * half:8 * half + 8, :]
                            .rearrange("p k q -> p q k"),
                            mybir.AxisListType.X, ALU.add)
                        s4[half] = pc.tile([P, QB], BF16, tag="ds4", bufs=4,
                                           name=f"ds4_{h}_{half}")
                        nc.vector.tensor_copy(s4[half][:], s4f[:])

                    pending = None
                    for kt in range(16):
                        psS = pss.tile([P, QB], F32, tag="s", name=f"psS{h}_{kt}")
                        nc.tensor.matmul(psS[:],
                                         KALL[:, h4::4 - h4,
                                              kt * P:(kt + 1) * P],
                                         Q8all[:, :, h, :],
                                         start=True, stop=True,
                                         perf_mode=DR_MODE)
                        nc.scalar.activation(PT[:, kt, :], psS[:], AFT.Exp,
                                             scale=SCALER)
                        if kt == 8:
                            den_half(0)
                        if pending is not None:
                            kp = pending
                            nc.tensor.matmul(psO[:], V_G[:, kp, h4 * P:(h4 + 1) * P],
                                             PT[:, kp, :],
                                             start=(kp == 0), stop=False)
                        pending = kt
                    kp = pending
                    nc.tensor.matmul(psO[:], V_G[:, kp, h4 * P:(h4 + 1) * P],
                                     PT[:, kp, :], start=False, stop=True)
                    den_half(1)

                    if pending_den:
                        flush_den(pending_den.pop(0))
                    pending_den.append((h, psO, s4[0], s4[1]))
            for item in pending_den:
                flush_den(item)

        # ---------- Phase D: partial out-projection ----------
        for nt in range(8):
            if nt == 0:
                wha, whb = wout_tiles
            else:
                wha = pd.tile([P, 8, 512], BF16, tag="wout", bufs=4,
                              name=f"wouta{nt}")
                nc.sync.dma_start(wha[:], t["wout"][nt, 0])
                whb = pd.tile([P, 8, 512], BF16, tag="wout", bufs=4,
                              name=f"woutb{nt}")
                nc.sync.dma_start(whb[:], t["wout"][nt, 1])
            for qt in range(4):
                ps = psg.tile([P, 512], F32, tag="psA", name=f"psOut{nt}_{qt}")
                for hh in range(GH):
                    w = wha if hh < 8 else whb
                    nc.tensor.matmul(ps[:],
                                     OT(hh)[:, qt * P:(qt + 1) * P],
                                     w[:, hh % 8, :],
                                     start=(hh == 0), stop=(hh == GH - 1))
                osb = pd.tile([P, 512], F16, tag="osb", bufs=3,
                              name=f"osb{nt}_{qt}")
                nc.scalar.activation(osb[:], ps[:], AFT.Copy)
                nc.sync.dma_start(
                    t["pout"][qt * P:(qt + 1) * P, nt * 512:(nt + 1) * 512],
                    osb[:])


def _build_program(rep=1):
    nc = bacc.Bacc("TRN2", target_bir_lowering=False, debug=False)
    t = {}
    t["seqT_my"] = nc.dram_tensor("t_seqT_my", [P, 32, QB], F8, kind="ExternalInput")
    t["seqT_mykb"] = nc.dram_tensor("t_seqT_mykb", [P, 32, 256], BF16, kind="ExternalInput")
    t["wdq"] = nc.dram_tensor("t_wdq", [12, P, 32, 128], F8, kind="ExternalInput")
    t["wdkv"] = nc.dram_tensor("t_wdkv", [P, 32, DC], BF16, kind="ExternalInput")
    t["wrk"] = nc.dram_tensor("t_wrk", [P, 32, DR], BF16, kind="ExternalInput")
    t["wuq"] = nc.dram_tensor("t_wuq", [GH, P, 12, DH], F8, kind="ExternalInput")
    t["wrq"] = nc.dram_tensor("t_wrq", [GH // 2, P, 12, 2 * DR], F8, kind="ExternalInput")
    t["wuk"] = nc.dram_tensor("t_wuk", [GH, P, 4, DH], F8, kind="ExternalInput")
    t["wuv"] = nc.dram_tensor("t_wuv", [4, P, 4, 512], BF16, kind="ExternalInput")
    t["wout"] = nc.dram_tensor("t_wout", [8, 2, P, 8, 512], BF16, kind="ExternalInput")
    t["bdq"] = nc.dram_tensor("t_bdq", [P, 12], F32, kind="ExternalInput")
    t["bdkv"] = nc.dram_tensor("t_bdkv", [P, 4], F32, kind="ExternalInput")
    t["brk"] = nc.dram_tensor("t_brk", [DR, 1], F32, kind="ExternalInput")
    t["buq"] = nc.dram_tensor("t_buq", [P, GH], F32, kind="ExternalInput")
    t["brq"] = nc.dram_tensor("t_brq", [P, GH // 2], F32, kind="ExternalInput")
    t["buk"] = nc.dram_tensor("t_buk", [P, GH], F32, kind="ExternalInput")
    t["buv2"] = nc.dram_tensor("t_buv2", [P, GH], F32, kind="ExternalInput")
    t["pout"] = nc.dram_tensor("t_pout", [QB, DM], F16, kind="ExternalOutput")

    with tile.TileContext(nc) as tc:
        for _ in range(rep):
            _emit_body(nc, tc, t)
    nc.compile()
    return nc


def _prep_shared(inputs):
    """Host-side layout + bf16 cast. Returns dict of shared arrays and
    per-head-group arrays."""
    bf = ml_dtypes.bfloat16
    f8 = ml_dtypes.float8_e4m3
    f32 = np.float32
    seq = np.asarray(inputs["sequence"], dtype=np.float32)[0]      # [2048, 4096]
    w_dq = np.asarray(inputs["w_dq"], dtype=np.float32)
    b_dq = np.asarray(inputs["b_dq"], dtype=np.float32)
    w_dkv = np.asarray(inputs["w_dkv"], dtype=np.float32)
    b_dkv = np.asarray(inputs["b_dkv"], dtype=np.float32)
    w_rk = np.asarray(inputs["w_rk"], dtype=np.float32)
    b_rk = np.asarray(inputs["b_rk"], dtype=np.float32)
    w_uq = np.asarray(inputs["w_uq"], dtype=np.float32)
    b_uq = np.asarray(inputs["b_uq"], dtype=np.float32)
    w_rq = np.asarray(inputs["w_rq"], dtype=np.float32)
    b_rq = np.asarray(inputs["b_rq"], dtype=np.float32)
    w_uk = np.asarray(inputs["w_uk"], dtype=np.float32)
    b_uk = np.asarray(inputs["b_uk"], dtype=np.float32)
    w_uv = np.asarray(inputs["w_uv"], dtype=np.float32)
    b_uv = np.asarray(inputs["b_uv"], dtype=np.float32)
    w_out = np.asarray(inputs["w_out"], dtype=np.float32)

    shared = {
        "seqT4": seq.reshape(4, 512, 32, P).transpose(0, 3, 2, 1).astype(f8),
        "seqT4b": seq.reshape(4, 512, 32, P).transpose(0, 3, 2, 1).astype(bf),
        "wdq": w_dq.reshape(32, P, 12, 128).transpose(2, 1, 0, 3).astype(f8),
        "wdkv": w_dkv.reshape(32, P, DC).transpose(1, 0, 2).astype(bf),
        "wrk": w_rk.reshape(32, P, DR).transpose(1, 0, 2).astype(bf),
        "bdq": np.ascontiguousarray(b_dq.reshape(12, P).T, dtype=f32),
        "bdkv": np.ascontiguousarray(b_dkv.reshape(4, P).T, dtype=f32),
        "brk": np.ascontiguousarray(b_rk.reshape(DR, 1), dtype=f32),
    }
    per_g = []
    for gi in range(NG):
        cols = slice(gi * GH * DH, (gi + 1) * GH * DH)       # 2048 cols
        c1k = slice(gi * GH * DR, (gi + 1) * GH * DR)        # 1024 cols
        per_g.append({
            "wuq": w_uq[:, cols].reshape(12, P, GH, DH).transpose(2, 1, 0, 3).astype(f8),
            "wrq": w_rq[:, c1k].reshape(12, P, GH // 2, 2 * DR).transpose(2, 1, 0, 3).astype(f8),
            "wuk": w_uk[:, cols].reshape(4, P, GH, DH).transpose(2, 1, 0, 3).astype(f8),
            "wuv": w_uv[:, cols].reshape(4, P, 4, 512).transpose(2, 1, 0, 3).astype(bf),
            "wout": w_out[cols, :].reshape(2, 8, P, 8, 512).transpose(3, 0, 2, 1, 4).astype(bf),
            "buv2": np.ascontiguousarray(b_uv[cols].reshape(GH, P).T, dtype=f32),
            "buq": np.ascontiguousarray(b_uq[cols].reshape(GH, P).T, dtype=f32),
            "brq": np.ascontiguousarray(b_rq[c1k].reshape(GH // 2, 2 * DR).T, dtype=f32),
            "buk": np.ascontiguousarray(b_uk[cols].reshape(GH, P).T, dtype=f32),
        })
    return shared, per_g


def _prep_in_maps(inputs):
    shared, per_g = _prep_shared(inputs)
    in_maps = []
    for c in range(8):
        gi, ri = c % NG, c // NG
        m = dict(shared)
        m.update(per_g[gi])
        m["seqT_my"] = np.ascontiguousarray(shared["seqT4"][ri])
        kb, half = c // 2, c % 2
        m["seqT_mykb"] = np.ascontiguousarray(
            shared["seqT4b"][kb][:, :, half * 256:(half + 1) * 256])
        del m["seqT4"]
        del m["seqT4b"]
        in_maps.append({f"t_{k}": v for k, v in m.items()})
    return in_maps


class _Runner:
    """Cached sharded PJRT executor for a compiled Bass program."""

    def __init__(self, nc):
        bass2jax.install_neuronx_cc_hook()
        self.nc = nc
        in_names, out_names, out_avals = [], [], []
        pid_name = nc.partition_id_tensor.name if nc.partition_id_tensor else None
        for alloc in nc.m.functions[0].allocations:
            if not isinstance(alloc, mybir.MemoryLocationSet):
                continue
            name = alloc.memorylocations[0].name
            if alloc.kind == "ExternalInput":
                if name != pid_name:
                    in_names.append(name)
            elif alloc.kind == "ExternalOutput":
                out_names.append(name)
                shape = tuple(alloc.tensor_shape)
                dtype = mybir.dt.np(alloc.dtype)
                out_avals.append(jax.core.ShapedArray(shape, dtype))
        self.in_names = in_names
        self.out_names = out_names
        all_in_names = list(in_names) + list(out_names)
        if pid_name is not None:
            all_in_names.append(pid_name)

        def _body(*args):
            operands = list(args)
            if nc.partition_id_tensor is not None:
                operands.append(bass2jax.partition_id_tensor())
            outs = bass2jax._bass_exec_p.bind(
                *operands,
                out_avals=tuple(out_avals),
                in_names=tuple(all_in_names),
                out_names=tuple(out_names),
                lowering_input_output_aliases=(),
                sim_require_finite=True,
                sim_require_nnan=True,
                nc=nc,
            )
            return tuple(outs)

        devices = jax.devices()[:8]
        self.mesh = Mesh(np.asarray(devices), ("core",))
        n_io = len(in_names) + len(out_names)
        self.fn = jax.jit(
            shard_map(_body, mesh=self.mesh,
                      in_specs=(PartitionSpec("core"),) * n_io,
                      out_specs=(PartitionSpec("core"),) * len(out_names),
                      check_rep=False),
            keep_unused=True)
        self.sharding = NamedSharding(self.mesh, PartitionSpec("core"))
        self.dev_zero = [
            jax.device_put(
                np.zeros((8 * av.shape[0], *av.shape[1:]), av.dtype), self.sharding)
            for av in out_avals]
        self.out_avals = out_avals

    def stage(self, in_maps):
        dev_in = []
        for name in self.in_names:
            cat = np.concatenate([np.asarray(m[name]) for m in in_maps], axis=0)
            dev_in.append(jax.device_put(cat, self.sharding))
        return dev_in

    def run_staged(self, dev_in):
        outs = self.fn(*dev_in, *self.dev_zero)
        jax.block_until_ready(outs)
        return outs

    def run(self, in_maps):
        outs = self.run_staged(self.stage(in_maps))
        res = []
        for c in range(8):
            d = {}
            for i, name in enumerate(self.out_names):
                av = self.out_avals[i]
                d[name] = np.asarray(outs[i]).reshape(8, *av.shape)[c]
            res.append(d)
        return res


_CTX = None


def _get_ctx():
    global _CTX
    if _CTX is None:
        nc = _build_program(rep=1)
        _CTX = _Runner(nc)
    return _CTX


def kernel(**inputs):
    runner = _get_ctx()
    in_maps = _prep_in_maps(inputs)
    res = runner.run(in_maps)
    b_out = np.asarray(inputs["b_out"], dtype=np.float32)
    out = np.empty((S, DM), dtype=np.float32)
    for ri in range(NR):
        acc = res[ri * NG + 0]["t_pout"].astype(np.float32)
        for gi in range(1, NG):
            acc += res[ri * NG + gi]["t_pout"].astype(np.float32)
        out[ri * QB:(ri + 1) * QB] = acc + b_out
    return out.reshape(1, S, DM)

